# revision 28
# baseline (speedup 1.0000x reference)
"""Head-sharded tensor-parallel causal attention for 8 TRN2 NeuronCores.

nn_Attention: B=1, S=4096, D_MODEL=1024, 16 heads (4 groups x 4), DK=64.
Reference returns (out, key, value) where key/value are the projected
K/V reshaped to [1, 4, 4, 4096, 64].

Sharding: 2 heads per core (columns c*128:(c+1)*128 of the QKV projections,
rows of Wo's contraction). Per core, everything is local:
  - Q.T, K.T computed as [128(dh), 4096(s)] via Wq/Wk column-shard matmuls
  - V computed as [4096(s), 128(dh)] (direct layout for the attn@V matmul)
  - scores S.T = K.T' @ Q.T per (key-block, q-chunk), exp on ACT with
    scale=1/8 fused; causal handled by skipping key-blocks above the
    diagonal + a tril mask multiply on the diagonal chunk
  - softmax denominator via a ones-column appended to V (matmul row-sums)
  - output projection vs Wo column-shard produces a per-core partial
    [4096, 1024]; the 8 partials are summed on the host (the "all-reduce"),
    bo added once on host.

Compute dtype: bf16 operands, fp32 PSUM accumulation. exp without max
subtraction (scores are bounded ~|9.4| for these inputs; exp fits fp32/bf16
easily).
"""

import numpy as np
import ml_dtypes

import concourse.bass as bass
import concourse.bacc as bacc
import concourse.tile as tile
from concourse import mybir
from concourse.bass_utils import run_bass_kernel_spmd

BF16 = ml_dtypes.bfloat16
S = 4096
D = 1024
DH = 64
NCORES = 8
SC = 512          # s-chunk (also q-chunk)
NSC = S // SC     # 8
NDT = D // 128    # 8 D-tiles
F32 = mybir.dt.float32
BF = mybir.dt.bfloat16
AF = mybir.ActivationFunctionType

# set by test.py to capture a profile
TRACE = False
LAST_RESULT = None
_CACHE = {}


def _build_nc():
    if "nc" in _CACHE:
        return _CACHE["nc"]
    nc = bacc.Bacc(
        "TRN2",
        target_bir_lowering=False,
        debug=False,
        enable_asserts=False,
        num_devices=NCORES,
    )

    xq = nc.dram_tensor("xq", [D, S], BF, kind="ExternalInput").ap()
    xk = nc.dram_tensor("xk", [D, S], BF, kind="ExternalInput").ap()
    xv = nc.dram_tensor("xv", [D, S], BF, kind="ExternalInput").ap()
    wq = nc.dram_tensor("wq", [D, 128], BF, kind="ExternalInput").ap()
    wk = nc.dram_tensor("wk", [D, 128], BF, kind="ExternalInput").ap()
    wv = nc.dram_tensor("wv", [D, 128], BF, kind="ExternalInput").ap()
    wo_dram = nc.dram_tensor("wo", [128, D], BF, kind="ExternalInput").ap()
    bq = nc.dram_tensor("bq", [128, 1], F32, kind="ExternalInput").ap()
    bk = nc.dram_tensor("bk", [128, 1], F32, kind="ExternalInput").ap()
    bv = nc.dram_tensor("bv", [128, 1], F32, kind="ExternalInput").ap()
    ident = nc.dram_tensor("ident", [128, 128], BF, kind="ExternalInput").ap()
    mask = nc.dram_tensor("mask", [128, 2, 128], BF, kind="ExternalInput").ap()

    out_p = nc.dram_tensor("out_p", [S, D], BF, kind="ExternalOutput").ap()
    key_s = nc.dram_tensor("key_s", [128, S], BF, kind="ExternalOutput").ap()
    val_s = nc.dram_tensor("val_s", [128, 32, 2, 65], BF,
                           kind="ExternalOutput").ap()

    with tile.TileContext(nc) as tc:
        import contextlib
        with contextlib.ExitStack() as ctx:
            const = ctx.enter_context(tc.tile_pool(name="const", bufs=1))
            persist = ctx.enter_context(tc.tile_pool(name="persist", bufs=1))
            stage = ctx.enter_context(tc.tile_pool(name="stage", bufs=10))
            work = ctx.enter_context(tc.tile_pool(name="work", bufs=3))
            epool = ctx.enter_context(tc.tile_pool(name="epool", bufs=8))
            psum = ctx.enter_context(
                tc.tile_pool(name="psum", bufs=2, space="PSUM"))

            # ---- constants ----
            # Q-path consts first so chunk-0 input DMAs start ASAP;
            # everything else is only needed a few us later
            wq_sb = const.tile([128, NDT, 128], BF, tag="wq")
            nc.sync.dma_start(wq_sb, wq.rearrange("(t p) m -> p t m", p=128))
            bq_sb = const.tile([128, 1], F32, tag="bq")
            nc.sync.dma_start(bq_sb, bq)
            wk_sb = const.tile([128, NDT, 128], BF, tag="wk")
            wv_sb = const.tile([128, NDT, 128], BF, tag="wv")
            wo_sb = const.tile([128, D], BF, tag="wo")
            bk_sb = const.tile([128, 1], F32, tag="bk")
            bv_sb = const.tile([128, 1], F32, tag="bv")
            ident_sb = const.tile([128, 128], BF, tag="ident")
            mask_sb = const.tile([128, 2, 128], BF, tag="mask")
            ones_sb = const.tile([1, 64], BF, tag="ones")

            def load_rest_consts():
                nc.sync.dma_start(wk_sb, wk.rearrange("(t p) m -> p t m", p=128))
                nc.sync.dma_start(wv_sb, wv.rearrange("(t p) m -> p t m", p=128))
                nc.sync.dma_start(wo_sb, wo_dram)
                nc.sync.dma_start(bk_sb, bk)
                nc.sync.dma_start(bv_sb, bv)
                nc.sync.dma_start(ident_sb, ident)
                nc.sync.dma_start(mask_sb, mask)
                nc.vector.memset(ones_sb, 1.0)

            # ---- persistent activations ----
            qT_sb = persist.tile([128, S], BF, tag="qT")
            kT_sb = persist.tile([128, S], BF, tag="kT")
            v_sb = persist.tile([128, 32, 2, 65], BF, tag="v")
            # ones column of V' (col 64 of each 65-group)
            nc.gpsimd.memset(v_sb[:, :, :, 64:65], 1.0)

            # input staging: one [128, 1024] DMA covers two s-chunks
            xcache = {}

            def xtile(name, xap, dt, sc):
                base = (sc // 2) * 2
                key = (name, dt, base)
                if key not in xcache:
                    t = stage.tile([128, 2 * SC], BF, tag=name,
                                   name=f"{name}{dt}_{base}")
                    nc.sync.dma_start(
                        t, xap[dt * 128:(dt + 1) * 128,
                               base * SC:(base + 2) * SC])
                    xcache[key] = t
                return xcache[key][:, (sc - base) * SC:(sc - base + 1) * SC]

            def proj_q(sc):
                ssl = slice(sc * SC, (sc + 1) * SC)
                ps_q = psum.tile([128, SC], F32, tag="mm")
                for dt in range(NDT):
                    qt = xtile("xq", xq, dt, sc)
                    nc.tensor.matmul(ps_q, wq_sb[:, dt, :], qt,
                                     start=(dt == 0), stop=(dt == NDT - 1))
                nc.vector.tensor_scalar_add(qT_sb[:, ssl], ps_q, bq_sb)

            def proj_kv(sc):
                ssl = slice(sc * SC, (sc + 1) * SC)
                ps_k = psum.tile([128, SC], F32, tag="mm")
                for dt in range(NDT):
                    kt = xtile("xk", xk, dt, sc)
                    nc.tensor.matmul(ps_k, wk_sb[:, dt, :], kt,
                                     start=(dt == 0), stop=(dt == NDT - 1))
                nc.vector.tensor_scalar_add(kT_sb[:, ssl], ps_k, bk_sb)
                nc.sync.dma_start(key_s[:, ssl], kT_sb[:, ssl])

                ps_vt = psum.tile([128, SC], F32, tag="mm")
                for dt in range(NDT):
                    vt = xtile("xv", xv, dt, sc)
                    nc.tensor.matmul(ps_vt, wv_sb[:, dt, :], vt,
                                     start=(dt == 0), stop=(dt == NDT - 1))
                vT_sb = work.tile([128, SC], BF, tag="vT")
                nc.vector.tensor_scalar_add(vT_sb, ps_vt, bv_sb)
                for t4 in range(4):
                    ps_tr = psum.tile([128, 128], BF, tag="mm")
                    nc.tensor.transpose(
                        ps_tr, vT_sb[:, t4 * 128:(t4 + 1) * 128], ident_sb)
                    nc.vector.tensor_copy(
                        v_sb[:, sc * 4 + t4, :, 0:64],
                        ps_tr.rearrange("p (h d) -> p h d", h=2))
                nc.sync.dma_start(val_s[:, sc * 4:(sc + 1) * 4, :, :],
                                  v_sb[:, sc * 4:(sc + 1) * 4, :, :])

            def attn_kb(sc, kb, nkb, ps_o):
                # diagonal key-blocks only cover queries q >= qlo; trim the
                # scores/exp/attnV column range and mask just the 128-wide
                # triangle block
                qlo = max(0, (kb - 4 * sc) * 128) if kb >= 4 * sc else 0
                q0 = sc * SC
                ps_s = psum.tile([128, 2, SC], F32, tag="s", name=f"ps_s_{sc}_{kb}")
                for h in range(2):
                    hsl = slice(64 * h, 64 * (h + 1))
                    nc.tensor.matmul(
                        ps_s[:, h, qlo:SC],
                        kT_sb[hsl, kb * 128:(kb + 1) * 128],
                        qT_sb[hsl, q0 + qlo:q0 + SC],
                        start=True, stop=True)
                exp_t = epool.tile([128, 2, SC], BF, tag="exp",
                                   name=f"exp_{sc}_{kb}")
                nc.scalar.activation(exp_t[:, :, qlo:SC], ps_s[:, :, qlo:SC],
                                     AF.Exp, scale=0.125)
                if kb >= 4 * sc:
                    nc.vector.tensor_mul(exp_t[:, :, qlo:qlo + 128],
                                         exp_t[:, :, qlo:qlo + 128], mask_sb)
                for h in range(2):
                    nc.tensor.matmul(
                        ps_o[h][:, qlo:SC], v_sb[:, kb, h, 0:65],
                        exp_t[:, h, qlo:SC],
                        start=(kb == 0), stop=(kb == nkb - 1))

            def attn_main(sc):
                # score/exp/attnV for keys of chunks 0..sc-1 (already ready)
                attnT = work.tile([128, SC], BF, tag="attnT")
                ps_o0 = psum.tile([65, SC], F32, tag="o")
                ps_o1 = psum.tile([65, SC], F32, tag="o")
                ps_o = [ps_o0, ps_o1]
                nkb = 4 * (sc + 1)
                for kb in range(4 * sc):
                    attn_kb(sc, kb, nkb, ps_o)
                return attnT, ps_o

            def attn_diag(sc, attnT, ps_o):
                # this chunk's own keys (needs proj_kv(sc)), then normalize
                nkb = 4 * (sc + 1)
                for kb in range(4 * sc, nkb):
                    attn_kb(sc, kb, nkb, ps_o)
                for h in range(2):
                    hsl = slice(64 * h, 64 * (h + 1))
                    sums = work.tile([1, SC], BF, tag="sums")
                    nc.vector.tensor_copy(sums, ps_o[h][64:65, :])
                    ps_b = psum.tile([64, SC], F32, tag="mm")
                    nc.tensor.matmul(ps_b, ones_sb, sums,
                                     start=True, stop=True)
                    bcast = work.tile([64, SC], F32, tag="bcast")
                    rscr = work.tile([64, SC], F32, tag="rscr")
                    nc.vector.reciprocal_approx_accurate(bcast, ps_b, rscr)
                    nc.vector.tensor_tensor(attnT[hsl, :], ps_o[h][0:64, :],
                                            bcast, mybir.AluOpType.mult)

            def wo(sc, attnT):
                for qsub in range(4):
                    o_sb = work.tile([128, 2, SC], BF, tag="osb")
                    for df in range(2):
                        ps_w = psum.tile([128, SC], F32, tag="mm",
                                         name=f"ps_w_{sc}_{qsub}_{df}")
                        nc.tensor.matmul(
                            ps_w,
                            attnT[:, qsub * 128:(qsub + 1) * 128],
                            wo_sb[:, df * SC:(df + 1) * SC],
                            start=True, stop=True)
                        nc.vector.tensor_copy(o_sb[:, df, :], ps_w)
                    r0 = sc * SC + qsub * 128
                    nc.sync.dma_start(out_p[r0:r0 + 128, :],
                                      o_sb.rearrange("p a b -> p (a b)"))

            # HAM warmup: keep the PE busy during the initial DMA ramp so
            # the clock gate opens before real matmuls start
            for w in range(14):
                ps_wu = psum.tile([128, SC], F32, tag="mm")
                nc.tensor.matmul(ps_wu, wq_sb[:, 0, :],
                                 wq_sb.rearrange("p t m -> p (t m)")[:, 0:SC],
                                 start=True, stop=True)

            # software pipeline: interleave so ACT always has exp backlog
            # while the PE runs projections, and Wo fills the norm latency
            proj_q(0)
            load_rest_consts()
            proj_kv(0)
            for sc in range(NSC):
                attnT, ps_o = attn_main(sc)
                if sc + 1 < NSC:
                    proj_q(sc + 1)
                attn_diag(sc, attnT, ps_o)
                if sc + 1 < NSC:
                    proj_kv(sc + 1)
                wo(sc, attnT)

    nc.compile()
    _CACHE["nc"] = nc
    return nc


def _prep_in_maps(q, k, v, Wq, bq, Wk, bk, Wv, bv, Wo):
    qT = np.ascontiguousarray(q[0].T).astype(BF16)   # [1024, 4096]
    kT = np.ascontiguousarray(k[0].T).astype(BF16)
    vT = np.ascontiguousarray(v[0].T).astype(BF16)

    kk = np.arange(128)[:, None, None]
    qq = np.arange(128)[None, None, :]
    mask = np.repeat((kk <= qq), 2, axis=1).astype(BF16)  # [128, 2, 128]

    in_maps = []
    for c in range(NCORES):
        sl = slice(c * 128, (c + 1) * 128)
        in_maps.append({
            "xq": qT, "xk": kT, "xv": vT,
            "wq": np.ascontiguousarray(Wq[sl, :].T).astype(BF16),
            "wk": np.ascontiguousarray(Wk[sl, :].T).astype(BF16),
            "wv": np.ascontiguousarray(Wv[sl, :].T).astype(BF16),
            "wo": np.ascontiguousarray(Wo[:, sl].T).astype(BF16),
            "bq": bq[sl].reshape(128, 1).astype(np.float32),
            "bk": bk[sl].reshape(128, 1).astype(np.float32),
            "bv": bv[sl].reshape(128, 1).astype(np.float32),
            "ident": np.eye(128, dtype=BF16),
            "mask": mask,
        })
    return in_maps


def kernel(q, k, v, Wq, bq, Wk, bk, Wv, bv, Wo, bo):
    global LAST_RESULT
    q, k, v = np.asarray(q), np.asarray(k), np.asarray(v)
    Wq, Wk, Wv, Wo = (np.asarray(x) for x in (Wq, Wk, Wv, Wo))
    bq, bk, bv, bo = (np.asarray(x) for x in (bq, bk, bv, bo))

    nc = _build_nc()
    in_maps = _prep_in_maps(q, k, v, Wq, bq, Wk, bk, Wv, bv, Wo)
    res = run_bass_kernel_spmd(nc, in_maps, core_ids=list(range(NCORES)),
                               trace=TRACE)
    LAST_RESULT = res
    rs = res.results

    out = np.zeros((S, D), np.float32)
    for c in range(NCORES):
        out += rs[c]["out_p"].astype(np.float32)
    out += bo[None, :].astype(np.float32)
    out = out[None]  # [1, 4096, 1024]

    key = np.stack([
        rs[c]["key_s"].astype(np.float32).reshape(2, DH, S).transpose(0, 2, 1)
        for c in range(NCORES)
    ]).reshape(1, 4, 4, S, DH)

    vals = []
    for c in range(NCORES):
        d = rs[c]["val_s"].astype(np.float32)[:, :, :, :64]  # [128,32,2,64]
        vals.append(np.transpose(d, (2, 1, 0, 3)).reshape(2, S, DH))
    val = np.stack(vals).reshape(1, 4, 4, S, DH)

    return out.astype(np.float32), key, val


# revision 29
# speedup vs baseline: 1.0050x; 1.0050x over previous
"""Head-sharded tensor-parallel causal attention for 8 TRN2 NeuronCores.

nn_Attention: B=1, S=4096, D_MODEL=1024, 16 heads (4 groups x 4), DK=64.
Reference returns (out, key, value) where key/value are the projected
K/V reshaped to [1, 4, 4, 4096, 64].

Sharding: 2 heads per core (columns c*128:(c+1)*128 of the QKV projections,
rows of Wo's contraction). Per core, everything is local:
  - Q.T, K.T computed as [128(dh), 4096(s)] via Wq/Wk column-shard matmuls
  - V computed as [4096(s), 128(dh)] (direct layout for the attn@V matmul)
  - scores S.T = K.T' @ Q.T per (key-block, q-chunk), exp on ACT with
    scale=1/8 fused; causal handled by skipping key-blocks above the
    diagonal + a tril mask multiply on the diagonal chunk
  - softmax denominator via a ones-column appended to V (matmul row-sums)
  - output projection vs Wo column-shard produces a per-core partial
    [4096, 1024]; the 8 partials are summed on the host (the "all-reduce"),
    bo added once on host.

Compute dtype: bf16 operands, fp32 PSUM accumulation. exp without max
subtraction (scores are bounded ~|9.4| for these inputs; exp fits fp32/bf16
easily).
"""

import numpy as np
import ml_dtypes

import concourse.bass as bass
import concourse.bacc as bacc
import concourse.tile as tile
from concourse import mybir
from concourse.bass_utils import run_bass_kernel_spmd

BF16 = ml_dtypes.bfloat16
S = 4096
D = 1024
DH = 64
NCORES = 8
SC = 512          # s-chunk (also q-chunk)
NSC = S // SC     # 8
NDT = D // 128    # 8 D-tiles
F32 = mybir.dt.float32
BF = mybir.dt.bfloat16
AF = mybir.ActivationFunctionType

# set by test.py to capture a profile
TRACE = False
LAST_RESULT = None
_CACHE = {}


def _build_nc():
    if "nc" in _CACHE:
        return _CACHE["nc"]
    nc = bacc.Bacc(
        "TRN2",
        target_bir_lowering=False,
        debug=False,
        enable_asserts=False,
        num_devices=NCORES,
    )

    xq = nc.dram_tensor("xq", [D, S], BF, kind="ExternalInput").ap()
    xk = nc.dram_tensor("xk", [D, S], BF, kind="ExternalInput").ap()
    xv = nc.dram_tensor("xv", [D, S], BF, kind="ExternalInput").ap()
    wq = nc.dram_tensor("wq", [D, 128], BF, kind="ExternalInput").ap()
    wk = nc.dram_tensor("wk", [D, 128], BF, kind="ExternalInput").ap()
    wv = nc.dram_tensor("wv", [D, 128], BF, kind="ExternalInput").ap()
    wo_dram = nc.dram_tensor("wo", [128, D], BF, kind="ExternalInput").ap()
    bq = nc.dram_tensor("bq", [128, 1], F32, kind="ExternalInput").ap()
    bk = nc.dram_tensor("bk", [128, 1], F32, kind="ExternalInput").ap()
    bv = nc.dram_tensor("bv", [128, 1], F32, kind="ExternalInput").ap()
    ident = nc.dram_tensor("ident", [128, 128], BF, kind="ExternalInput").ap()
    mask = nc.dram_tensor("mask", [128, 2, 128], BF, kind="ExternalInput").ap()

    out_p = nc.dram_tensor("out_p", [S, D], BF, kind="ExternalOutput").ap()
    key_s = nc.dram_tensor("key_s", [128, S], BF, kind="ExternalOutput").ap()
    val_s = nc.dram_tensor("val_s", [128, 32, 2, 65], BF,
                           kind="ExternalOutput").ap()

    with tile.TileContext(nc) as tc:
        import contextlib
        with contextlib.ExitStack() as ctx:
            const = ctx.enter_context(tc.tile_pool(name="const", bufs=1))
            persist = ctx.enter_context(tc.tile_pool(name="persist", bufs=1))
            stage = ctx.enter_context(tc.tile_pool(name="stage", bufs=10))
            work = ctx.enter_context(tc.tile_pool(name="work", bufs=3))
            epool = ctx.enter_context(tc.tile_pool(name="epool", bufs=8))
            psum = ctx.enter_context(
                tc.tile_pool(name="psum", bufs=2, space="PSUM"))

            # ---- constants ----
            # Q-path consts first so chunk-0 input DMAs start ASAP;
            # everything else is only needed a few us later
            wq_sb = const.tile([128, NDT, 128], BF, tag="wq")
            nc.sync.dma_start(wq_sb, wq.rearrange("(t p) m -> p t m", p=128))
            bq_sb = const.tile([128, 1], F32, tag="bq")
            nc.sync.dma_start(bq_sb, bq)
            wk_sb = const.tile([128, NDT, 128], BF, tag="wk")
            wv_sb = const.tile([128, NDT, 128], BF, tag="wv")
            wo_sb = const.tile([128, D], BF, tag="wo")
            bk_sb = const.tile([128, 1], F32, tag="bk")
            bv_sb = const.tile([128, 1], F32, tag="bv")
            ident_sb = const.tile([128, 128], BF, tag="ident")
            mask_sb = const.tile([128, 2, 128], BF, tag="mask")
            ones_sb = const.tile([1, 64], BF, tag="ones")

            def load_rest_consts():
                nc.sync.dma_start(wk_sb, wk.rearrange("(t p) m -> p t m", p=128))
                nc.sync.dma_start(wv_sb, wv.rearrange("(t p) m -> p t m", p=128))
                nc.sync.dma_start(wo_sb, wo_dram)
                nc.sync.dma_start(bk_sb, bk)
                nc.sync.dma_start(bv_sb, bv)
                nc.sync.dma_start(ident_sb, ident)
                nc.sync.dma_start(mask_sb, mask)
                nc.vector.memset(ones_sb, 1.0)

            # ---- persistent activations ----
            qT_sb = persist.tile([128, S], BF, tag="qT")
            kT_sb = persist.tile([128, S], BF, tag="kT")
            v_sb = persist.tile([128, 32, 2, 65], BF, tag="v")
            # ones column of V' (col 64 of each 65-group)
            nc.gpsimd.memset(v_sb[:, :, :, 64:65], 1.0)

            # input staging: one [128, 1024] DMA covers two s-chunks
            xcache = {}

            def xtile(name, xap, dt, sc):
                base = (sc // 2) * 2
                key = (name, dt, base)
                if key not in xcache:
                    t = stage.tile([128, 2 * SC], BF, tag=name,
                                   name=f"{name}{dt}_{base}")
                    nc.sync.dma_start(
                        t, xap[dt * 128:(dt + 1) * 128,
                               base * SC:(base + 2) * SC])
                    xcache[key] = t
                return xcache[key][:, (sc - base) * SC:(sc - base + 1) * SC]

            def proj_q(sc):
                ssl = slice(sc * SC, (sc + 1) * SC)
                ps_q = psum.tile([128, SC], F32, tag="mm")
                for dt in range(NDT):
                    qt = xtile("xq", xq, dt, sc)
                    nc.tensor.matmul(ps_q, wq_sb[:, dt, :], qt,
                                     start=(dt == 0), stop=(dt == NDT - 1))
                nc.vector.tensor_scalar_add(qT_sb[:, ssl], ps_q, bq_sb)

            def proj_kv(sc):
                ssl = slice(sc * SC, (sc + 1) * SC)
                ps_k = psum.tile([128, SC], F32, tag="mm")
                for dt in range(NDT):
                    kt = xtile("xk", xk, dt, sc)
                    nc.tensor.matmul(ps_k, wk_sb[:, dt, :], kt,
                                     start=(dt == 0), stop=(dt == NDT - 1))
                nc.vector.tensor_scalar_add(kT_sb[:, ssl], ps_k, bk_sb)
                nc.sync.dma_start(key_s[:, ssl], kT_sb[:, ssl])

                ps_vt = psum.tile([128, SC], F32, tag="mm")
                for dt in range(NDT):
                    vt = xtile("xv", xv, dt, sc)
                    nc.tensor.matmul(ps_vt, wv_sb[:, dt, :], vt,
                                     start=(dt == 0), stop=(dt == NDT - 1))
                vT_sb = work.tile([128, SC], BF, tag="vT")
                nc.vector.tensor_scalar_add(vT_sb, ps_vt, bv_sb)
                for t4 in range(4):
                    ps_tr = psum.tile([128, 128], BF, tag="mm")
                    nc.tensor.transpose(
                        ps_tr, vT_sb[:, t4 * 128:(t4 + 1) * 128], ident_sb)
                    nc.vector.tensor_copy(
                        v_sb[:, sc * 4 + t4, :, 0:64],
                        ps_tr.rearrange("p (h d) -> p h d", h=2))
                nc.sync.dma_start(val_s[:, sc * 4:(sc + 1) * 4, :, :],
                                  v_sb[:, sc * 4:(sc + 1) * 4, :, :])

            def attn_kb(sc, kb, nkb, ps_o):
                # diagonal key-blocks only cover queries q >= qlo; trim the
                # scores/exp/attnV column range and mask just the 128-wide
                # triangle block
                qlo = max(0, (kb - 4 * sc) * 128) if kb >= 4 * sc else 0
                q0 = sc * SC
                ps_s = psum.tile([128, 2, SC], F32, tag="s", name=f"ps_s_{sc}_{kb}")
                for h in range(2):
                    hsl = slice(64 * h, 64 * (h + 1))
                    nc.tensor.matmul(
                        ps_s[:, h, qlo:SC],
                        kT_sb[hsl, kb * 128:(kb + 1) * 128],
                        qT_sb[hsl, q0 + qlo:q0 + SC],
                        start=True, stop=True)
                exp_t = epool.tile([128, 2, SC], BF, tag="exp",
                                   name=f"exp_{sc}_{kb}")
                nc.scalar.activation(exp_t[:, :, qlo:SC], ps_s[:, :, qlo:SC],
                                     AF.Exp, scale=0.125)
                if kb >= 4 * sc:
                    nc.vector.tensor_mul(exp_t[:, :, qlo:qlo + 128],
                                         exp_t[:, :, qlo:qlo + 128], mask_sb)
                for h in range(2):
                    nc.tensor.matmul(
                        ps_o[h][:, qlo:SC], v_sb[:, kb, h, 0:65],
                        exp_t[:, h, qlo:SC],
                        start=(kb == 0), stop=(kb == nkb - 1))

            def attn_main(sc):
                # score/exp/attnV for keys of chunks 0..sc-1 (already ready)
                attnT = work.tile([128, SC], BF, tag="attnT")
                ps_o0 = psum.tile([65, SC], F32, tag="o")
                ps_o1 = psum.tile([65, SC], F32, tag="o")
                ps_o = [ps_o0, ps_o1]
                nkb = 4 * (sc + 1)
                for kb in range(4 * sc):
                    attn_kb(sc, kb, nkb, ps_o)
                return attnT, ps_o

            def attn_diag(sc, attnT, ps_o):
                # this chunk's own keys (needs proj_kv(sc)), then normalize
                nkb = 4 * (sc + 1)
                for kb in range(4 * sc, nkb):
                    attn_kb(sc, kb, nkb, ps_o)
                for h in range(2):
                    hsl = slice(64 * h, 64 * (h + 1))
                    sums = work.tile([1, SC], BF, tag="sums")
                    nc.scalar.copy(sums, ps_o[h][64:65, :])
                    ps_b = psum.tile([64, SC], F32, tag="mm")
                    nc.tensor.matmul(ps_b, ones_sb, sums,
                                     start=True, stop=True)
                    bcast = work.tile([64, SC], F32, tag="bcast")
                    rscr = work.tile([64, SC], F32, tag="rscr")
                    nc.vector.reciprocal_approx_accurate(bcast, ps_b, rscr)
                    nc.vector.tensor_tensor(attnT[hsl, :], ps_o[h][0:64, :],
                                            bcast, mybir.AluOpType.mult)

            def wo(sc, attnT):
                for qsub in range(4):
                    o_sb = work.tile([128, 2, SC], BF, tag="osb")
                    for df in range(2):
                        ps_w = psum.tile([128, SC], F32, tag="mm",
                                         name=f"ps_w_{sc}_{qsub}_{df}")
                        nc.tensor.matmul(
                            ps_w,
                            attnT[:, qsub * 128:(qsub + 1) * 128],
                            wo_sb[:, df * SC:(df + 1) * SC],
                            start=True, stop=True)
                        nc.vector.tensor_copy(o_sb[:, df, :], ps_w)
                    r0 = sc * SC + qsub * 128
                    nc.sync.dma_start(out_p[r0:r0 + 128, :],
                                      o_sb.rearrange("p a b -> p (a b)"))

            # HAM warmup: keep the PE busy during the initial DMA ramp so
            # the clock gate opens before real matmuls start
            for w in range(14):
                ps_wu = psum.tile([128, SC], F32, tag="mm")
                nc.tensor.matmul(ps_wu, wq_sb[:, 0, :],
                                 wq_sb.rearrange("p t m -> p (t m)")[:, 0:SC],
                                 start=True, stop=True)

            # software pipeline: interleave so ACT always has exp backlog
            # while the PE runs projections, and Wo fills the norm latency
            proj_q(0)
            load_rest_consts()
            proj_kv(0)
            for sc in range(NSC):
                attnT, ps_o = attn_main(sc)
                if sc + 1 < NSC:
                    proj_q(sc + 1)
                attn_diag(sc, attnT, ps_o)
                if sc + 1 < NSC:
                    proj_kv(sc + 1)
                wo(sc, attnT)

    nc.compile()
    _CACHE["nc"] = nc
    return nc


def _prep_in_maps(q, k, v, Wq, bq, Wk, bk, Wv, bv, Wo):
    qT = np.ascontiguousarray(q[0].T).astype(BF16)   # [1024, 4096]
    kT = np.ascontiguousarray(k[0].T).astype(BF16)
    vT = np.ascontiguousarray(v[0].T).astype(BF16)

    kk = np.arange(128)[:, None, None]
    qq = np.arange(128)[None, None, :]
    mask = np.repeat((kk <= qq), 2, axis=1).astype(BF16)  # [128, 2, 128]

    in_maps = []
    for c in range(NCORES):
        sl = slice(c * 128, (c + 1) * 128)
        in_maps.append({
            "xq": qT, "xk": kT, "xv": vT,
            "wq": np.ascontiguousarray(Wq[sl, :].T).astype(BF16),
            "wk": np.ascontiguousarray(Wk[sl, :].T).astype(BF16),
            "wv": np.ascontiguousarray(Wv[sl, :].T).astype(BF16),
            "wo": np.ascontiguousarray(Wo[:, sl].T).astype(BF16),
            "bq": bq[sl].reshape(128, 1).astype(np.float32),
            "bk": bk[sl].reshape(128, 1).astype(np.float32),
            "bv": bv[sl].reshape(128, 1).astype(np.float32),
            "ident": np.eye(128, dtype=BF16),
            "mask": mask,
        })
    return in_maps


def kernel(q, k, v, Wq, bq, Wk, bk, Wv, bv, Wo, bo):
    global LAST_RESULT
    q, k, v = np.asarray(q), np.asarray(k), np.asarray(v)
    Wq, Wk, Wv, Wo = (np.asarray(x) for x in (Wq, Wk, Wv, Wo))
    bq, bk, bv, bo = (np.asarray(x) for x in (bq, bk, bv, bo))

    nc = _build_nc()
    in_maps = _prep_in_maps(q, k, v, Wq, bq, Wk, bk, Wv, bv, Wo)
    res = run_bass_kernel_spmd(nc, in_maps, core_ids=list(range(NCORES)),
                               trace=TRACE)
    LAST_RESULT = res
    rs = res.results

    out = np.zeros((S, D), np.float32)
    for c in range(NCORES):
        out += rs[c]["out_p"].astype(np.float32)
    out += bo[None, :].astype(np.float32)
    out = out[None]  # [1, 4096, 1024]

    key = np.stack([
        rs[c]["key_s"].astype(np.float32).reshape(2, DH, S).transpose(0, 2, 1)
        for c in range(NCORES)
    ]).reshape(1, 4, 4, S, DH)

    vals = []
    for c in range(NCORES):
        d = rs[c]["val_s"].astype(np.float32)[:, :, :, :64]  # [128,32,2,64]
        vals.append(np.transpose(d, (2, 1, 0, 3)).reshape(2, S, DH))
    val = np.stack(vals).reshape(1, 4, 4, S, DH)

    return out.astype(np.float32), key, val


# revision 30
# speedup vs baseline: 1.0263x; 1.0211x over previous
"""Head-sharded tensor-parallel causal attention for 8 TRN2 NeuronCores.

nn_Attention: B=1, S=4096, D_MODEL=1024, 16 heads (4 groups x 4), DK=64.
Reference returns (out, key, value) where key/value are the projected
K/V reshaped to [1, 4, 4, 4096, 64].

Sharding: 2 heads per core (columns c*128:(c+1)*128 of the QKV projections,
rows of Wo's contraction). Per core, everything is local:
  - Q.T, K.T computed as [128(dh), 4096(s)] via Wq/Wk column-shard matmuls
  - V computed as [4096(s), 128(dh)] (direct layout for the attn@V matmul)
  - scores S.T = K.T' @ Q.T per (key-block, q-chunk), exp on ACT with
    scale=1/8 fused; causal handled by skipping key-blocks above the
    diagonal + a tril mask multiply on the diagonal chunk
  - softmax denominator via a ones-column appended to V (matmul row-sums)
  - output projection vs Wo column-shard produces a per-core partial
    [4096, 1024]; the 8 partials are summed on the host (the "all-reduce"),
    bo added once on host.

Compute dtype: bf16 operands, fp32 PSUM accumulation. exp without max
subtraction (scores are bounded ~|9.4| for these inputs; exp fits fp32/bf16
easily).
"""

import numpy as np
import ml_dtypes

import concourse.bass as bass
import concourse.bacc as bacc
import concourse.tile as tile
from concourse import mybir
from concourse.bass_utils import run_bass_kernel_spmd

BF16 = ml_dtypes.bfloat16
S = 4096
D = 1024
DH = 64
NCORES = 8
SC = 512          # s-chunk (also q-chunk)
NSC = S // SC     # 8
NDT = D // 128    # 8 D-tiles
F32 = mybir.dt.float32
BF = mybir.dt.bfloat16
AF = mybir.ActivationFunctionType

# set by test.py to capture a profile
TRACE = False
LAST_RESULT = None
_CACHE = {}


def _build_nc():
    if "nc" in _CACHE:
        return _CACHE["nc"]
    nc = bacc.Bacc(
        "TRN2",
        target_bir_lowering=False,
        debug=False,
        enable_asserts=False,
        num_devices=NCORES,
    )

    xq = nc.dram_tensor("xq", [D, S], BF, kind="ExternalInput").ap()
    xk = nc.dram_tensor("xk", [D, S], BF, kind="ExternalInput").ap()
    xv = nc.dram_tensor("xv", [D, S], BF, kind="ExternalInput").ap()
    wq = nc.dram_tensor("wq", [D, 128], BF, kind="ExternalInput").ap()
    wk = nc.dram_tensor("wk", [D, 128], BF, kind="ExternalInput").ap()
    wv = nc.dram_tensor("wv", [D, 128], BF, kind="ExternalInput").ap()
    wo_dram = nc.dram_tensor("wo", [128, D], BF, kind="ExternalInput").ap()
    bq = nc.dram_tensor("bq", [128, 1], F32, kind="ExternalInput").ap()
    bk = nc.dram_tensor("bk", [128, 1], F32, kind="ExternalInput").ap()
    bv = nc.dram_tensor("bv", [128, 1], F32, kind="ExternalInput").ap()
    ident = nc.dram_tensor("ident", [128, 128], BF, kind="ExternalInput").ap()
    mask = nc.dram_tensor("mask", [128, 2, 128], BF, kind="ExternalInput").ap()

    out_p = nc.dram_tensor("out_p", [S, D], BF, kind="ExternalOutput").ap()
    key_s = nc.dram_tensor("key_s", [128, S], BF, kind="ExternalOutput").ap()
    val_s = nc.dram_tensor("val_s", [128, 32, 2, 65], BF,
                           kind="ExternalOutput").ap()

    with tile.TileContext(nc) as tc:
        import contextlib
        with contextlib.ExitStack() as ctx:
            const = ctx.enter_context(tc.tile_pool(name="const", bufs=1))
            persist = ctx.enter_context(tc.tile_pool(name="persist", bufs=1))
            stage = ctx.enter_context(tc.tile_pool(name="stage", bufs=10))
            work = ctx.enter_context(tc.tile_pool(name="work", bufs=3))
            epool = ctx.enter_context(tc.tile_pool(name="epool", bufs=8))
            psum = ctx.enter_context(
                tc.tile_pool(name="psum", bufs=2, space="PSUM"))

            # ---- constants ----
            # Q-path consts first so chunk-0 input DMAs start ASAP;
            # everything else is only needed a few us later
            wq_sb = const.tile([128, NDT, 128], BF, tag="wq")
            nc.sync.dma_start(wq_sb, wq.rearrange("(t p) m -> p t m", p=128))
            bq_sb = const.tile([128, 1], F32, tag="bq")
            nc.sync.dma_start(bq_sb, bq)
            wk_sb = const.tile([128, NDT, 128], BF, tag="wk")
            wv_sb = const.tile([128, NDT, 128], BF, tag="wv")
            wo_sb = const.tile([128, D], BF, tag="wo")
            bk_sb = const.tile([128, 1], F32, tag="bk")
            bv_sb = const.tile([128, 1], F32, tag="bv")
            ident_sb = const.tile([128, 128], BF, tag="ident")
            mask_sb = const.tile([128, 2, 128], BF, tag="mask")
            ones_sb = const.tile([1, 64], BF, tag="ones")

            def load_rest_consts():
                nc.sync.dma_start(wk_sb, wk.rearrange("(t p) m -> p t m", p=128))
                nc.sync.dma_start(wv_sb, wv.rearrange("(t p) m -> p t m", p=128))
                nc.sync.dma_start(wo_sb, wo_dram)
                nc.sync.dma_start(bk_sb, bk)
                nc.sync.dma_start(bv_sb, bv)
                nc.sync.dma_start(ident_sb, ident)
                nc.sync.dma_start(mask_sb, mask)
                nc.vector.memset(ones_sb, 1.0)

            # ---- persistent activations ----
            qT_sb = persist.tile([128, S], BF, tag="qT")
            kT_sb = persist.tile([128, S], BF, tag="kT")
            v_sb = persist.tile([128, 32, 2, 65], BF, tag="v")
            # ones column of V' (col 64 of each 65-group)
            nc.gpsimd.memset(v_sb[:, :, :, 64:65], 1.0)

            # input staging: one [128, 1024] DMA covers two s-chunks
            xcache = {}

            def xtile(name, xap, dt, sc):
                base = (sc // 2) * 2
                key = (name, dt, base)
                if key not in xcache:
                    t = stage.tile([128, 2 * SC], BF, tag=name,
                                   name=f"{name}{dt}_{base}")
                    nc.sync.dma_start(
                        t, xap[dt * 128:(dt + 1) * 128,
                               base * SC:(base + 2) * SC])
                    xcache[key] = t
                return xcache[key][:, (sc - base) * SC:(sc - base + 1) * SC]

            def proj_q(sc):
                ssl = slice(sc * SC, (sc + 1) * SC)
                ps_q = psum.tile([128, SC], F32, tag="mm")
                for dt in range(NDT):
                    qt = xtile("xq", xq, dt, sc)
                    nc.tensor.matmul(ps_q, wq_sb[:, dt, :], qt,
                                     start=(dt == 0), stop=(dt == NDT - 1))
                nc.vector.tensor_scalar_add(qT_sb[:, ssl], ps_q, bq_sb)

            def proj_kv(sc):
                ssl = slice(sc * SC, (sc + 1) * SC)
                ps_k = psum.tile([128, SC], F32, tag="mm")
                for dt in range(NDT):
                    kt = xtile("xk", xk, dt, sc)
                    nc.tensor.matmul(ps_k, wk_sb[:, dt, :], kt,
                                     start=(dt == 0), stop=(dt == NDT - 1))
                nc.vector.tensor_scalar_add(kT_sb[:, ssl], ps_k, bk_sb)
                nc.sync.dma_start(key_s[:, ssl], kT_sb[:, ssl])

                ps_vt = psum.tile([128, SC], F32, tag="mm")
                for dt in range(NDT):
                    vt = xtile("xv", xv, dt, sc)
                    nc.tensor.matmul(ps_vt, wv_sb[:, dt, :], vt,
                                     start=(dt == 0), stop=(dt == NDT - 1))
                vT_sb = work.tile([128, SC], BF, tag="vT")
                nc.vector.tensor_scalar_add(vT_sb, ps_vt, bv_sb)
                for t4 in range(4):
                    ps_tr = psum.tile([128, 128], BF, tag="mm")
                    nc.tensor.transpose(
                        ps_tr, vT_sb[:, t4 * 128:(t4 + 1) * 128], ident_sb)
                    nc.vector.tensor_copy(
                        v_sb[:, sc * 4 + t4, :, 0:64],
                        ps_tr.rearrange("p (h d) -> p h d", h=2))
                nc.sync.dma_start(val_s[:, sc * 4:(sc + 1) * 4, :, :],
                                  v_sb[:, sc * 4:(sc + 1) * 4, :, :])

            def attn_kb(sc, kb, nkb, ps_o):
                # diagonal key-blocks only cover queries q >= qlo; trim the
                # scores/exp/attnV column range and mask just the 128-wide
                # triangle block
                qlo = max(0, (kb - 4 * sc) * 128) if kb >= 4 * sc else 0
                q0 = sc * SC
                ps_s = psum.tile([128, 2, SC], F32, tag="s", name=f"ps_s_{sc}_{kb}")
                for h in range(2):
                    hsl = slice(64 * h, 64 * (h + 1))
                    nc.tensor.matmul(
                        ps_s[:, h, qlo:SC],
                        kT_sb[hsl, kb * 128:(kb + 1) * 128],
                        qT_sb[hsl, q0 + qlo:q0 + SC],
                        start=True, stop=True)
                exp_t = epool.tile([128, 2, SC], BF, tag="exp",
                                   name=f"exp_{sc}_{kb}")
                nc.scalar.activation(exp_t[:, :, qlo:SC], ps_s[:, :, qlo:SC],
                                     AF.Exp, scale=0.125)
                if kb >= 4 * sc:
                    nc.vector.tensor_mul(exp_t[:, :, qlo:qlo + 128],
                                         exp_t[:, :, qlo:qlo + 128], mask_sb)
                for h in range(2):
                    nc.tensor.matmul(
                        ps_o[h][:, qlo:SC], v_sb[:, kb, h, 0:65],
                        exp_t[:, h, qlo:SC],
                        start=(kb == 0), stop=(kb == nkb - 1))

            def attn_main(sc):
                # score/exp/attnV for keys of chunks 0..sc-1 (already ready)
                attnT = work.tile([128, SC], BF, tag="attnT")
                ps_o0 = psum.tile([65, SC], F32, tag="o")
                ps_o1 = psum.tile([65, SC], F32, tag="o")
                ps_o = [ps_o0, ps_o1]
                nkb = 4 * (sc + 1)
                for kb in range(4 * sc):
                    attn_kb(sc, kb, nkb, ps_o)
                return attnT, ps_o

            def attn_diag(sc, attnT, ps_o):
                # this chunk's own keys (needs proj_kv(sc)), then normalize
                nkb = 4 * (sc + 1)
                for kb in range(4 * sc, nkb):
                    attn_kb(sc, kb, nkb, ps_o)
                for h in range(2):
                    hsl = slice(64 * h, 64 * (h + 1))
                    sums = work.tile([1, SC], BF, tag="sums")
                    nc.scalar.copy(sums, ps_o[h][64:65, :])
                    ps_b = psum.tile([64, SC], F32, tag="s")
                    nc.tensor.matmul(ps_b, ones_sb, sums,
                                     start=True, stop=True)
                    bcast = work.tile([64, SC], F32, tag="bcast")
                    rscr = work.tile([64, SC], F32, tag="rscr")
                    nc.vector.reciprocal_approx_accurate(bcast, ps_b, rscr)
                    nc.vector.tensor_tensor(attnT[hsl, :], ps_o[h][0:64, :],
                                            bcast, mybir.AluOpType.mult)

            def wo(sc, attnT):
                for qsub in range(4):
                    o_sb = work.tile([128, 2, SC], BF, tag="osb")
                    for df in range(2):
                        ps_w = psum.tile([128, SC], F32, tag="mm",
                                         name=f"ps_w_{sc}_{qsub}_{df}")
                        nc.tensor.matmul(
                            ps_w,
                            attnT[:, qsub * 128:(qsub + 1) * 128],
                            wo_sb[:, df * SC:(df + 1) * SC],
                            start=True, stop=True)
                        nc.vector.tensor_copy(o_sb[:, df, :], ps_w)
                    r0 = sc * SC + qsub * 128
                    nc.sync.dma_start(out_p[r0:r0 + 128, :],
                                      o_sb.rearrange("p a b -> p (a b)"))

            # HAM warmup: keep the PE busy during the initial DMA ramp so
            # the clock gate opens before real matmuls start
            for w in range(14):
                ps_wu = psum.tile([128, SC], F32, tag="mm")
                nc.tensor.matmul(ps_wu, wq_sb[:, 0, :],
                                 wq_sb.rearrange("p t m -> p (t m)")[:, 0:SC],
                                 start=True, stop=True)

            # software pipeline: interleave so ACT always has exp backlog
            # while the PE runs projections, and Wo fills the norm latency
            proj_q(0)
            load_rest_consts()
            proj_kv(0)
            for sc in range(NSC):
                attnT, ps_o = attn_main(sc)
                if sc + 1 < NSC:
                    proj_q(sc + 1)
                attn_diag(sc, attnT, ps_o)
                if sc + 1 < NSC:
                    proj_kv(sc + 1)
                wo(sc, attnT)

    nc.compile()
    _CACHE["nc"] = nc
    return nc


def _prep_in_maps(q, k, v, Wq, bq, Wk, bk, Wv, bv, Wo):
    qT = np.ascontiguousarray(q[0].T).astype(BF16)   # [1024, 4096]
    kT = np.ascontiguousarray(k[0].T).astype(BF16)
    vT = np.ascontiguousarray(v[0].T).astype(BF16)

    kk = np.arange(128)[:, None, None]
    qq = np.arange(128)[None, None, :]
    mask = np.repeat((kk <= qq), 2, axis=1).astype(BF16)  # [128, 2, 128]

    in_maps = []
    for c in range(NCORES):
        sl = slice(c * 128, (c + 1) * 128)
        in_maps.append({
            "xq": qT, "xk": kT, "xv": vT,
            "wq": np.ascontiguousarray(Wq[sl, :].T).astype(BF16),
            "wk": np.ascontiguousarray(Wk[sl, :].T).astype(BF16),
            "wv": np.ascontiguousarray(Wv[sl, :].T).astype(BF16),
            "wo": np.ascontiguousarray(Wo[:, sl].T).astype(BF16),
            "bq": bq[sl].reshape(128, 1).astype(np.float32),
            "bk": bk[sl].reshape(128, 1).astype(np.float32),
            "bv": bv[sl].reshape(128, 1).astype(np.float32),
            "ident": np.eye(128, dtype=BF16),
            "mask": mask,
        })
    return in_maps


def kernel(q, k, v, Wq, bq, Wk, bk, Wv, bv, Wo, bo):
    global LAST_RESULT
    q, k, v = np.asarray(q), np.asarray(k), np.asarray(v)
    Wq, Wk, Wv, Wo = (np.asarray(x) for x in (Wq, Wk, Wv, Wo))
    bq, bk, bv, bo = (np.asarray(x) for x in (bq, bk, bv, bo))

    nc = _build_nc()
    in_maps = _prep_in_maps(q, k, v, Wq, bq, Wk, bk, Wv, bv, Wo)
    res = run_bass_kernel_spmd(nc, in_maps, core_ids=list(range(NCORES)),
                               trace=TRACE)
    LAST_RESULT = res
    rs = res.results

    out = np.zeros((S, D), np.float32)
    for c in range(NCORES):
        out += rs[c]["out_p"].astype(np.float32)
    out += bo[None, :].astype(np.float32)
    out = out[None]  # [1, 4096, 1024]

    key = np.stack([
        rs[c]["key_s"].astype(np.float32).reshape(2, DH, S).transpose(0, 2, 1)
        for c in range(NCORES)
    ]).reshape(1, 4, 4, S, DH)

    vals = []
    for c in range(NCORES):
        d = rs[c]["val_s"].astype(np.float32)[:, :, :, :64]  # [128,32,2,64]
        vals.append(np.transpose(d, (2, 1, 0, 3)).reshape(2, S, DH))
    val = np.stack(vals).reshape(1, 4, 4, S, DH)

    return out.astype(np.float32), key, val


# revision 31
# speedup vs baseline: 1.0612x; 1.0340x over previous
"""Head-sharded tensor-parallel causal attention for 8 TRN2 NeuronCores.

nn_Attention: B=1, S=4096, D_MODEL=1024, 16 heads (4 groups x 4), DK=64.
Reference returns (out, key, value) where key/value are the projected
K/V reshaped to [1, 4, 4, 4096, 64].

Sharding: 2 heads per core (columns c*128:(c+1)*128 of the QKV projections,
rows of Wo's contraction). Per core, everything is local:
  - Q.T, K.T computed as [128(dh), 4096(s)] via Wq/Wk column-shard matmuls
  - V computed as [4096(s), 128(dh)] (direct layout for the attn@V matmul)
  - scores S.T = K.T' @ Q.T per (key-block, q-chunk), exp on ACT with
    scale=1/8 fused; causal handled by skipping key-blocks above the
    diagonal + a tril mask multiply on the diagonal chunk
  - softmax denominator via a ones-column appended to V (matmul row-sums)
  - output projection vs Wo column-shard produces a per-core partial
    [4096, 1024]; the 8 partials are summed on the host (the "all-reduce"),
    bo added once on host.

Compute dtype: bf16 operands, fp32 PSUM accumulation. exp without max
subtraction (scores are bounded ~|9.4| for these inputs; exp fits fp32/bf16
easily).
"""

import numpy as np
import ml_dtypes

import concourse.bass as bass
import concourse.bacc as bacc
import concourse.tile as tile
from concourse import mybir
from concourse.bass_utils import run_bass_kernel_spmd

BF16 = ml_dtypes.bfloat16
S = 4096
D = 1024
DH = 64
NCORES = 8
SC = 512          # s-chunk (also q-chunk)
NSC = S // SC     # 8
NDT = D // 128    # 8 D-tiles
F32 = mybir.dt.float32
BF = mybir.dt.bfloat16
AF = mybir.ActivationFunctionType

# set by test.py to capture a profile
TRACE = False
LAST_RESULT = None
_CACHE = {}


def _build_nc():
    if "nc" in _CACHE:
        return _CACHE["nc"]
    nc = bacc.Bacc(
        "TRN2",
        target_bir_lowering=False,
        debug=False,
        enable_asserts=False,
        num_devices=NCORES,
    )

    xq = nc.dram_tensor("xq", [D, S], BF, kind="ExternalInput").ap()
    xk = nc.dram_tensor("xk", [D, S], BF, kind="ExternalInput").ap()
    xv = nc.dram_tensor("xv", [D, S], BF, kind="ExternalInput").ap()
    wq = nc.dram_tensor("wq", [D, 128], BF, kind="ExternalInput").ap()
    wk = nc.dram_tensor("wk", [D, 128], BF, kind="ExternalInput").ap()
    wv = nc.dram_tensor("wv", [D, 128], BF, kind="ExternalInput").ap()
    wo_dram = nc.dram_tensor("wo", [128, D], BF, kind="ExternalInput").ap()
    bq = nc.dram_tensor("bq", [128, 1], F32, kind="ExternalInput").ap()
    bk = nc.dram_tensor("bk", [128, 1], F32, kind="ExternalInput").ap()
    bv = nc.dram_tensor("bv", [128, 1], F32, kind="ExternalInput").ap()
    ident = nc.dram_tensor("ident", [128, 128], BF, kind="ExternalInput").ap()
    mask = nc.dram_tensor("mask", [128, 2, 128], BF, kind="ExternalInput").ap()

    out_p = nc.dram_tensor("out_p", [S, D], BF, kind="ExternalOutput").ap()
    key_s = nc.dram_tensor("key_s", [128, S], BF, kind="ExternalOutput").ap()
    val_s = nc.dram_tensor("val_s", [128, 32, 2, 65], BF,
                           kind="ExternalOutput").ap()

    with tile.TileContext(nc) as tc:
        import contextlib
        with contextlib.ExitStack() as ctx:
            const = ctx.enter_context(tc.tile_pool(name="const", bufs=1))
            persist = ctx.enter_context(tc.tile_pool(name="persist", bufs=1))
            stage = ctx.enter_context(tc.tile_pool(name="stage", bufs=10))
            work = ctx.enter_context(tc.tile_pool(name="work", bufs=4))
            epool = ctx.enter_context(tc.tile_pool(name="epool", bufs=8))
            psum = ctx.enter_context(
                tc.tile_pool(name="psum", bufs=2, space="PSUM"))

            # ---- constants ----
            # Q-path consts first so chunk-0 input DMAs start ASAP;
            # everything else is only needed a few us later
            wq_sb = const.tile([128, NDT, 128], BF, tag="wq")
            nc.sync.dma_start(wq_sb, wq.rearrange("(t p) m -> p t m", p=128))
            bq_sb = const.tile([128, 1], F32, tag="bq")
            nc.sync.dma_start(bq_sb, bq)
            wk_sb = const.tile([128, NDT, 128], BF, tag="wk")
            wv_sb = const.tile([128, NDT, 128], BF, tag="wv")
            wo_sb = const.tile([128, D], BF, tag="wo")
            bk_sb = const.tile([128, 1], F32, tag="bk")
            bv_sb = const.tile([128, 1], F32, tag="bv")
            ident_sb = const.tile([128, 128], BF, tag="ident")
            mask_sb = const.tile([128, 2, 128], BF, tag="mask")
            ones_sb = const.tile([1, 64], BF, tag="ones")

            def load_rest_consts():
                nc.sync.dma_start(wk_sb, wk.rearrange("(t p) m -> p t m", p=128))
                nc.sync.dma_start(wv_sb, wv.rearrange("(t p) m -> p t m", p=128))
                nc.sync.dma_start(wo_sb, wo_dram)
                nc.sync.dma_start(bk_sb, bk)
                nc.sync.dma_start(bv_sb, bv)
                nc.sync.dma_start(ident_sb, ident)
                nc.sync.dma_start(mask_sb, mask)
                nc.vector.memset(ones_sb, 1.0)

            # ---- persistent activations ----
            qT_sb = persist.tile([128, S], BF, tag="qT")
            kT_sb = persist.tile([128, S], BF, tag="kT")
            v_sb = persist.tile([128, 32, 2, 65], BF, tag="v")
            # ones column of V' (col 64 of each 65-group)
            nc.gpsimd.memset(v_sb[:, :, :, 64:65], 1.0)

            # input staging: one [128, 1024] DMA covers two s-chunks
            xcache = {}

            def xtile(name, xap, dt, sc):
                base = (sc // 2) * 2
                key = (name, dt, base)
                if key not in xcache:
                    t = stage.tile([128, 2 * SC], BF, tag=name,
                                   name=f"{name}{dt}_{base}")
                    nc.sync.dma_start(
                        t, xap[dt * 128:(dt + 1) * 128,
                               base * SC:(base + 2) * SC])
                    xcache[key] = t
                return xcache[key][:, (sc - base) * SC:(sc - base + 1) * SC]

            def proj_q(sc):
                ssl = slice(sc * SC, (sc + 1) * SC)
                ps_q = psum.tile([128, SC], F32, tag="mm")
                for dt in range(NDT):
                    qt = xtile("xq", xq, dt, sc)
                    nc.tensor.matmul(ps_q, wq_sb[:, dt, :], qt,
                                     start=(dt == 0), stop=(dt == NDT - 1))
                nc.vector.tensor_scalar_add(qT_sb[:, ssl], ps_q, bq_sb)

            def proj_kv(sc):
                ssl = slice(sc * SC, (sc + 1) * SC)
                ps_k = psum.tile([128, SC], F32, tag="mm")
                for dt in range(NDT):
                    kt = xtile("xk", xk, dt, sc)
                    nc.tensor.matmul(ps_k, wk_sb[:, dt, :], kt,
                                     start=(dt == 0), stop=(dt == NDT - 1))
                nc.vector.tensor_scalar_add(kT_sb[:, ssl], ps_k, bk_sb)
                nc.sync.dma_start(key_s[:, ssl], kT_sb[:, ssl])

                ps_vt = psum.tile([128, SC], F32, tag="mm")
                for dt in range(NDT):
                    vt = xtile("xv", xv, dt, sc)
                    nc.tensor.matmul(ps_vt, wv_sb[:, dt, :], vt,
                                     start=(dt == 0), stop=(dt == NDT - 1))
                vT_sb = work.tile([128, SC], BF, tag="vT")
                nc.vector.tensor_scalar_add(vT_sb, ps_vt, bv_sb)
                for t4 in range(4):
                    ps_tr = psum.tile([128, 128], BF, tag="mm")
                    nc.tensor.transpose(
                        ps_tr, vT_sb[:, t4 * 128:(t4 + 1) * 128], ident_sb)
                    nc.vector.tensor_copy(
                        v_sb[:, sc * 4 + t4, :, 0:64],
                        ps_tr.rearrange("p (h d) -> p h d", h=2))
                nc.sync.dma_start(val_s[:, sc * 4:(sc + 1) * 4, :, :],
                                  v_sb[:, sc * 4:(sc + 1) * 4, :, :])

            def attn_kb(sc, kb, nkb, ps_o):
                # diagonal key-blocks only cover queries q >= qlo; trim the
                # scores/exp/attnV column range and mask just the 128-wide
                # triangle block
                qlo = max(0, (kb - 4 * sc) * 128) if kb >= 4 * sc else 0
                q0 = sc * SC
                ps_s = psum.tile([128, 2, SC], F32, tag="s", name=f"ps_s_{sc}_{kb}")
                for h in range(2):
                    hsl = slice(64 * h, 64 * (h + 1))
                    nc.tensor.matmul(
                        ps_s[:, h, qlo:SC],
                        kT_sb[hsl, kb * 128:(kb + 1) * 128],
                        qT_sb[hsl, q0 + qlo:q0 + SC],
                        start=True, stop=True)
                exp_t = epool.tile([128, 2, SC], BF, tag="exp",
                                   name=f"exp_{sc}_{kb}")
                nc.scalar.activation(exp_t[:, :, qlo:SC], ps_s[:, :, qlo:SC],
                                     AF.Exp, scale=0.125)
                if kb >= 4 * sc:
                    nc.vector.tensor_mul(exp_t[:, :, qlo:qlo + 128],
                                         exp_t[:, :, qlo:qlo + 128], mask_sb)
                for h in range(2):
                    nc.tensor.matmul(
                        ps_o[h][:, qlo:SC], v_sb[:, kb, h, 0:65],
                        exp_t[:, h, qlo:SC],
                        start=(kb == 0), stop=(kb == nkb - 1))

            def attn_main(sc):
                # score/exp/attnV for keys of chunks 0..sc-1 (already ready)
                attnT = work.tile([128, SC], BF, tag="attnT")
                ps_o0 = psum.tile([65, SC], F32, tag="o")
                ps_o1 = psum.tile([65, SC], F32, tag="o")
                ps_o = [ps_o0, ps_o1]
                nkb = 4 * (sc + 1)
                for kb in range(4 * sc):
                    attn_kb(sc, kb, nkb, ps_o)
                return attnT, ps_o

            def attn_diag(sc, attnT, ps_o):
                # this chunk's own keys (needs proj_kv(sc)), then normalize
                nkb = 4 * (sc + 1)
                for kb in range(4 * sc, nkb):
                    attn_kb(sc, kb, nkb, ps_o)
                for h in range(2):
                    hsl = slice(64 * h, 64 * (h + 1))
                    sums = work.tile([1, SC], BF, tag="sums")
                    nc.scalar.copy(sums, ps_o[h][64:65, :])
                    ps_b = psum.tile([64, SC], F32, tag="s")
                    nc.tensor.matmul(ps_b, ones_sb, sums,
                                     start=True, stop=True)
                    bcast = work.tile([64, SC], F32, tag="bcast")
                    rscr = work.tile([64, SC], F32, tag="rscr")
                    nc.vector.reciprocal_approx_accurate(bcast, ps_b, rscr)
                    nc.vector.tensor_tensor(attnT[hsl, :], ps_o[h][0:64, :],
                                            bcast, mybir.AluOpType.mult)

            def wo(sc, attnT):
                for qsub in range(4):
                    o_sb = work.tile([128, 2, SC], BF, tag="osb")
                    for df in range(2):
                        ps_w = psum.tile([128, SC], F32, tag="mm",
                                         name=f"ps_w_{sc}_{qsub}_{df}")
                        nc.tensor.matmul(
                            ps_w,
                            attnT[:, qsub * 128:(qsub + 1) * 128],
                            wo_sb[:, df * SC:(df + 1) * SC],
                            start=True, stop=True)
                        nc.vector.tensor_copy(o_sb[:, df, :], ps_w)
                    r0 = sc * SC + qsub * 128
                    nc.sync.dma_start(out_p[r0:r0 + 128, :],
                                      o_sb.rearrange("p a b -> p (a b)"))

            # HAM warmup: keep the PE busy during the initial DMA ramp so
            # the clock gate opens before real matmuls start
            for w in range(14):
                ps_wu = psum.tile([128, SC], F32, tag="mm")
                nc.tensor.matmul(ps_wu, wq_sb[:, 0, :],
                                 wq_sb.rearrange("p t m -> p (t m)")[:, 0:SC],
                                 start=True, stop=True)

            # software pipeline: interleave so ACT always has exp backlog
            # while the PE runs projections, and Wo fills the norm latency
            proj_q(0)
            load_rest_consts()
            proj_kv(0)
            for sc in range(NSC):
                attnT, ps_o = attn_main(sc)
                if sc + 1 < NSC:
                    proj_q(sc + 1)
                attn_diag(sc, attnT, ps_o)
                if sc + 1 < NSC:
                    proj_kv(sc + 1)
                wo(sc, attnT)

    nc.compile()
    _CACHE["nc"] = nc
    return nc


def _prep_in_maps(q, k, v, Wq, bq, Wk, bk, Wv, bv, Wo):
    qT = np.ascontiguousarray(q[0].T).astype(BF16)   # [1024, 4096]
    kT = np.ascontiguousarray(k[0].T).astype(BF16)
    vT = np.ascontiguousarray(v[0].T).astype(BF16)

    kk = np.arange(128)[:, None, None]
    qq = np.arange(128)[None, None, :]
    mask = np.repeat((kk <= qq), 2, axis=1).astype(BF16)  # [128, 2, 128]

    in_maps = []
    for c in range(NCORES):
        sl = slice(c * 128, (c + 1) * 128)
        in_maps.append({
            "xq": qT, "xk": kT, "xv": vT,
            "wq": np.ascontiguousarray(Wq[sl, :].T).astype(BF16),
            "wk": np.ascontiguousarray(Wk[sl, :].T).astype(BF16),
            "wv": np.ascontiguousarray(Wv[sl, :].T).astype(BF16),
            "wo": np.ascontiguousarray(Wo[:, sl].T).astype(BF16),
            "bq": bq[sl].reshape(128, 1).astype(np.float32),
            "bk": bk[sl].reshape(128, 1).astype(np.float32),
            "bv": bv[sl].reshape(128, 1).astype(np.float32),
            "ident": np.eye(128, dtype=BF16),
            "mask": mask,
        })
    return in_maps


def kernel(q, k, v, Wq, bq, Wk, bk, Wv, bv, Wo, bo):
    global LAST_RESULT
    q, k, v = np.asarray(q), np.asarray(k), np.asarray(v)
    Wq, Wk, Wv, Wo = (np.asarray(x) for x in (Wq, Wk, Wv, Wo))
    bq, bk, bv, bo = (np.asarray(x) for x in (bq, bk, bv, bo))

    nc = _build_nc()
    in_maps = _prep_in_maps(q, k, v, Wq, bq, Wk, bk, Wv, bv, Wo)
    res = run_bass_kernel_spmd(nc, in_maps, core_ids=list(range(NCORES)),
                               trace=TRACE)
    LAST_RESULT = res
    rs = res.results

    out = np.zeros((S, D), np.float32)
    for c in range(NCORES):
        out += rs[c]["out_p"].astype(np.float32)
    out += bo[None, :].astype(np.float32)
    out = out[None]  # [1, 4096, 1024]

    key = np.stack([
        rs[c]["key_s"].astype(np.float32).reshape(2, DH, S).transpose(0, 2, 1)
        for c in range(NCORES)
    ]).reshape(1, 4, 4, S, DH)

    vals = []
    for c in range(NCORES):
        d = rs[c]["val_s"].astype(np.float32)[:, :, :, :64]  # [128,32,2,64]
        vals.append(np.transpose(d, (2, 1, 0, 3)).reshape(2, S, DH))
    val = np.stack(vals).reshape(1, 4, 4, S, DH)

    return out.astype(np.float32), key, val
